# revision 18
# baseline (speedup 1.0000x reference)
"""ColorDiversityLoss kernel for Trainium2 (8 NeuronCores, Bass/Tile).

Math: pixels p[b] = generated[b].reshape(3, N).T  (N = 96*96 = 9216, 3 channels)
      dist[b][i, j] = || p[i] - p[j] ||_2   (torch.cdist p=2 semantics)
      out = -mean over (b, column j, k=8) of the 8 smallest dist[b][:, j]

The distance matrix is symmetric, so "8 smallest per column over rows" ==
"8 smallest per row over columns".  Sharding: 2 batches x 4 row-chunks ->
8 cores, each core handles 2304 rows x all 9216 columns, flash-style
(the N x N matrix never exists in HBM).

Per core, for each 128-row tile:
  - TensorE computes  v = -sq = 2*q.p - |p_c|^2 - |q_r|^2  for all 9216
    columns directly in PSUM via a K=16 bf16 matmul (fp32 pixels are split
    hi/lo into two bf16 factors; all 4 cross products are kept, and the
    squared-norm terms ride along as extra contraction rows), accurate to
    ~1e-6 absolute.
  - ScalarE evicts PSUM -> SBUF.
  - VectorE `max` (the HW top-8 instruction) finds the 8 largest v per row
    == 8 smallest squared distances, in one pass.
The [2304, 8] candidates per core are DMA'd out; the host applies
sqrt/clamp and the mean.  The top candidate of row r is always the r==c
diagonal (|v| ~ 1e-6 vs ~ -2.5e-3 for the nearest real neighbor), whose
true distance is exactly 0; the host drops it and substitutes 0,
reproducing the reference's exact-zero diagonal.
"""

import os
import numpy as np
import ml_dtypes

BF16 = ml_dtypes.bfloat16

B = 2
C = 3
N = 9216                 # 96*96 pixels per batch element
N_CORES = 8
CHUNKS = 4               # row-chunks per batch element
ROWS = N // CHUNKS       # 2304 rows per core
TILE_P = 128
N_TILES = ROWS // TILE_P  # 18
KDIM = 16                # contraction rows of the hi/lo matmul
MM_N = 512               # one PSUM bank of fp32
PSUM_COLS = 2048         # 4 banks per psum tile
TOPK = 8

_CACHE = {}

LAST_RESULTS = None


N_EVICT = 7168           # columns evicted to SBUF fp16 by ScalarE per tile
N_DIRECT = N - N_EVICT   # columns consumed directly from PSUM by max8
EV_CHUNK = 1024          # 2-bank PSUM tiles, bufs=2 -> fine-grained rotation
assert N_EVICT % EV_CHUNK == 0


def _build_program():
    """v2: per 128-row tile, split the 9216 columns:
      - cols [0, 8192): PE fills PSUM, ScalarE evicts to SBUF as bf16,
        VectorE folds twice with tensor-tensor max (2x mode) then max8
        over the remaining 2048  -> candB (bf16).
      - cols [8192, 9216): VectorE max8 straight from PSUM -> candA (fp32).
    Host merges the two candidate lists."""
    from contextlib import ExitStack
    from concourse import bacc, tile, mybir

    nc = bacc.Bacc("TRN2", target_bir_lowering=False, debug=False,
                   enable_asserts=False)

    lhsT_d = nc.dram_tensor("lhsT", [KDIM, ROWS], mybir.dt.bfloat16,
                            kind="ExternalInput").ap()
    rhs_d = nc.dram_tensor("rhs", [KDIM, N], mybir.dt.bfloat16,
                           kind="ExternalInput").ap()
    candA_d = nc.dram_tensor("candA", [ROWS, 2 * TOPK], mybir.dt.float32,
                             kind="ExternalOutput").ap()
    candB_d = nc.dram_tensor("candB", [ROWS, TOPK], mybir.dt.float16,
                             kind="ExternalOutput").ap()

    with tile.TileContext(nc) as tc:
        with ExitStack() as ctx:
            const = ctx.enter_context(tc.tile_pool(name="const", bufs=1))
            ev_psum_pool = ctx.enter_context(
                tc.tile_pool(name="ev_psum", bufs=2, space="PSUM"))
            dir_psum_pool = ctx.enter_context(
                tc.tile_pool(name="dir_psum", bufs=2, space="PSUM"))
            dist_pool = ctx.enter_context(tc.tile_pool(name="dist", bufs=2))
            f1_pool = ctx.enter_context(tc.tile_pool(name="f1", bufs=2))
            f2_pool = ctx.enter_context(tc.tile_pool(name="f2", bufs=2))
            cand_pool = ctx.enter_context(tc.tile_pool(name="cand", bufs=3))

            qT = const.tile([KDIM, ROWS], mybir.dt.bfloat16)
            pT = const.tile([KDIM, N], mybir.dt.bfloat16)
            # ordered + split across two trigger queues so tile 0's
            # operands land first: sync carries qT-head + the direct-part
            # columns, gpsimd carries the bulk
            nc.sync.dma_start(qT[:, :TILE_P], lhsT_d[:, :TILE_P])
            nc.sync.dma_start(pT[:, N_EVICT:], rhs_d[:, N_EVICT:])
            nc.gpsimd.dma_start(qT[:, TILE_P:], lhsT_d[:, TILE_P:])
            for c in range(0, N_EVICT, 1792):
                nc.sync.dma_start(pT[:, c:c + 1792], rhs_d[:, c:c + 1792])

            DIR_CHUNK = N_DIRECT // 2

            def emit_direct(t, lhs_tile):
                # direct part: cols [N_EVICT, N) -> max8 straight from PSUM,
                # as two double-buffered 2-bank tiles so PE is never blocked
                # behind a pending max8.
                for j in range(2):
                    psum_dir = dir_psum_pool.tile([TILE_P, DIR_CHUNK],
                                                  mybir.dt.float32, tag="dir")
                    c0 = N_EVICT + j * DIR_CHUNK
                    for b in range(0, DIR_CHUNK, MM_N):
                        nc.tensor.matmul(
                            psum_dir[:, b:b + MM_N],
                            lhs_tile,
                            pT[:, c0 + b:c0 + b + MM_N],
                            start=True, stop=True)
                    candA = cand_pool.tile([TILE_P, TOPK], mybir.dt.float32,
                                           tag="candA")
                    nc.vector.max(out=candA[:], in_=psum_dir[:])
                    nc.sync.dma_start(
                        candA_d[t * TILE_P:(t + 1) * TILE_P,
                                j * TOPK:(j + 1) * TOPK],
                        candA[:])

            for t in range(N_TILES):
                lhs_tile = qT[:, t * TILE_P:(t + 1) * TILE_P]

                if t == 0:
                    # tile 0: direct part first so VectorE has work while
                    # the eviction pipeline fills
                    emit_direct(t, lhs_tile)

                # evicted part: cols [0, N_EVICT), ScalarE drains PSUM->fp16
                dist = dist_pool.tile([TILE_P, N_EVICT], mybir.dt.float16)
                for c in range(0, N_EVICT, EV_CHUNK):
                    psum = ev_psum_pool.tile([TILE_P, EV_CHUNK],
                                             mybir.dt.float32, tag="ps")
                    for b in range(0, EV_CHUNK, MM_N):
                        nc.tensor.matmul(
                            psum[:, b:b + MM_N],
                            lhs_tile,
                            pT[:, c + b:c + b + MM_N],
                            start=True, stop=True)
                    nc.scalar.activation(
                        dist[:, c:c + EV_CHUNK], psum[:],
                        mybir.ActivationFunctionType.Copy)

                if t > 0:
                    emit_direct(t, lhs_tile)

                candB = cand_pool.tile([TILE_P, TOPK], mybir.dt.float16,
                                       tag="candB")
                # chunk-pair fold tree: starts as soon as two chunks are
                # evicted; max 4 source columns per folded slot (same
                # collision budget as a half-fold chain), contiguous 2048
                # final max8, and only one small TT + the max8 after the
                # final eviction.
                ck = EV_CHUNK
                tr = f1_pool.tile([TILE_P, 4 * ck], mybir.dt.float16,
                                  tag="tree")
                nc.vector.tensor_max(tr[:, 2 * ck:3 * ck],
                                     dist[:, 0:ck], dist[:, ck:2 * ck])
                nc.vector.tensor_max(tr[:, 3 * ck:4 * ck],
                                     dist[:, 2 * ck:3 * ck],
                                     dist[:, 3 * ck:4 * ck])
                nc.vector.tensor_max(tr[:, 0:ck],
                                     tr[:, 2 * ck:3 * ck],
                                     tr[:, 3 * ck:4 * ck])
                nc.vector.tensor_max(tr[:, 2 * ck:3 * ck],
                                     dist[:, 4 * ck:5 * ck],
                                     dist[:, 5 * ck:6 * ck])
                nc.vector.tensor_max(tr[:, ck:2 * ck],
                                     tr[:, 2 * ck:3 * ck],
                                     dist[:, 6 * ck:7 * ck])
                nc.vector.max(out=candB[:], in_=tr[:, 0:2 * ck])
                nc.sync.dma_start(candB_d[t * TILE_P:(t + 1) * TILE_P, :],
                                  candB[:])

    nc.compile()
    return nc


def _split_hi_lo(x32):
    """fp32 array -> (hi, lo) bf16 pair with hi + lo ~= x to ~18 bits."""
    hi = x32.astype(BF16)
    lo = (x32 - hi.astype(np.float32)).astype(BF16)
    return hi, lo


def _prep_batch(p):
    """p: [N, 3] float32 pixels -> (lhsT_full [16, N], rhs [16, N]) bf16."""
    ph, pl = _split_hi_lo(p)                      # [N, 3] each
    p64 = ph.astype(np.float64) + pl.astype(np.float64)
    sqn = np.einsum("nd,nd->n", p64, p64)         # [N] float64
    snh = sqn.astype(BF16)
    snl = (sqn - snh.astype(np.float64)).astype(np.float32).astype(BF16)

    rhs = np.empty((KDIM, N), BF16)
    lhsT = np.empty((KDIM, N), BF16)
    for d in range(C):
        two_ph = (2.0 * ph[:, d].astype(np.float32)).astype(BF16)
        two_pl = (2.0 * pl[:, d].astype(np.float32)).astype(BF16)
        rhs[4 * d + 0] = two_ph
        rhs[4 * d + 1] = two_pl
        rhs[4 * d + 2] = two_ph
        rhs[4 * d + 3] = two_pl
        lhsT[4 * d + 0] = ph[:, d]
        lhsT[4 * d + 1] = ph[:, d]
        lhsT[4 * d + 2] = pl[:, d]
        lhsT[4 * d + 3] = pl[:, d]
    one = np.ones(N, BF16)
    rhs[12] = -snh
    rhs[13] = -snl
    rhs[14] = one
    rhs[15] = one
    lhsT[12] = one
    lhsT[13] = one
    lhsT[14] = -snh
    lhsT[15] = -snl
    return lhsT, rhs


def _enable_tracing():
    """Best-effort NTFF tracing under axon: install the missing
    antenv.axon_hooks shim and disable the artifact upload."""
    import sys
    import types
    try:
        import antenv.axon_hooks  # noqa: F401
    except ImportError:
        try:
            import antenv
            from trn_agent_boot.trn_boot import _ntff_profile_via_ctypes
            hook = _ntff_profile_via_ctypes("/opt/axon/libaxon_pjrt.so")
            mod = types.ModuleType("antenv.axon_hooks")
            state = {"hook": hook}
            mod.get_axon_ntff_profile_hook = lambda: state["hook"]
            mod.set_axon_ntff_profile_hook = (
                lambda h: state.__setitem__("hook", h))
            sys.modules["antenv.axon_hooks"] = mod
            antenv.axon_hooks = mod
        except Exception as e:  # tracing is optional
            print(f"tracing hook unavailable: {e}")
            return False
    from concourse import bass_utils
    bass_utils.upload_artifacts = lambda tmpdir: f"local://{tmpdir}"
    return True


def kernel(generated) -> np.ndarray:
    global LAST_RESULTS
    from concourse.bass_utils import run_bass_kernel_spmd

    if "nc" not in _CACHE:
        _CACHE["nc"] = _build_program()
    nc = _CACHE["nc"]

    g = np.asarray(generated).astype(np.float32)
    assert g.shape == (B, C, 96, 96), g.shape
    pixels = g.reshape(B, C, N).transpose(0, 2, 1)  # [B, N, 3]

    per_batch = [_prep_batch(np.ascontiguousarray(pixels[b]))
                 for b in range(B)]

    in_maps = []
    for core in range(N_CORES):
        b, ch = divmod(core, CHUNKS)
        lhsT_full, rhs = per_batch[b]
        in_maps.append({
            "lhsT": np.ascontiguousarray(
                lhsT_full[:, ch * ROWS:(ch + 1) * ROWS]),
            "rhs": rhs,
        })

    trace = bool(os.environ.get("KERNEL_TRACE"))
    if trace:
        trace = _enable_tracing()
    res = run_bass_kernel_spmd(
        nc, in_maps, list(range(N_CORES)),
        trace=trace,
        tmpdir=os.environ.get("KERNEL_TRACE_DIR") or None)
    LAST_RESULTS = res

    candA = np.stack([res.results[i]["candA"] for i in range(N_CORES)])
    candB = np.stack([res.results[i]["candB"].astype(np.float32)
                      for i in range(N_CORES)])
    # candA: [8, 2304, 16] (two direct chunks), candB: [8, 2304, 8]; all
    # -sq, descending per row.  Merge, take the global top 8 per row;
    # slot 0 is the diagonal (true value 0).
    cand = np.concatenate([candA, candB], axis=2)          # [8, 2304, 24]
    cand = -np.sort(-cand.astype(np.float64), axis=2)[:, :, :TOPK]
    sq = np.maximum(-cand, 0.0)
    d = np.sqrt(sq)
    total = d[:, :, 1:TOPK].sum()   # diagonal contributes exactly 0
    mean = total / (B * N * TOPK)
    return np.float32(-mean)


# revision 19
# speedup vs baseline: 1.0007x; 1.0007x over previous
"""ColorDiversityLoss kernel for Trainium2 (8 NeuronCores, Bass/Tile).

Math: pixels p[b] = generated[b].reshape(3, N).T  (N = 96*96 = 9216, 3 channels)
      dist[b][i, j] = || p[i] - p[j] ||_2   (torch.cdist p=2 semantics)
      out = -mean over (b, column j, k=8) of the 8 smallest dist[b][:, j]

The distance matrix is symmetric, so "8 smallest per column over rows" ==
"8 smallest per row over columns".  Sharding: 2 batches x 4 row-chunks ->
8 cores, each core handles 2304 rows x all 9216 columns, flash-style
(the N x N matrix never exists in HBM).

Per core, for each 128-row tile:
  - TensorE computes  v = -sq = 2*q.p - |p_c|^2 - |q_r|^2  for all 9216
    columns directly in PSUM via a K=16 bf16 matmul (fp32 pixels are split
    hi/lo into two bf16 factors; all 4 cross products are kept, and the
    squared-norm terms ride along as extra contraction rows), accurate to
    ~1e-6 absolute.
  - ScalarE evicts PSUM -> SBUF.
  - VectorE `max` (the HW top-8 instruction) finds the 8 largest v per row
    == 8 smallest squared distances, in one pass.
The [2304, 8] candidates per core are DMA'd out; the host applies
sqrt/clamp and the mean.  The top candidate of row r is always the r==c
diagonal (|v| ~ 1e-6 vs ~ -2.5e-3 for the nearest real neighbor), whose
true distance is exactly 0; the host drops it and substitutes 0,
reproducing the reference's exact-zero diagonal.
"""

import os
import numpy as np
import ml_dtypes

BF16 = ml_dtypes.bfloat16

B = 2
C = 3
N = 9216                 # 96*96 pixels per batch element
N_CORES = 8
CHUNKS = 4               # row-chunks per batch element
ROWS = N // CHUNKS       # 2304 rows per core
TILE_P = 128
N_TILES = ROWS // TILE_P  # 18
KDIM = 16                # contraction rows of the hi/lo matmul
MM_N = 512               # one PSUM bank of fp32
PSUM_COLS = 2048         # 4 banks per psum tile
TOPK = 8

_CACHE = {}

LAST_RESULTS = None


N_EVICT = 7168           # columns evicted to SBUF fp16 by ScalarE per tile
N_DIRECT = N - N_EVICT   # columns consumed directly from PSUM by max8
EV_CHUNK = 1024          # 2-bank PSUM tiles, bufs=2 -> fine-grained rotation
assert N_EVICT % EV_CHUNK == 0


def _build_program():
    """v2: per 128-row tile, split the 9216 columns:
      - cols [0, 8192): PE fills PSUM, ScalarE evicts to SBUF as bf16,
        VectorE folds twice with tensor-tensor max (2x mode) then max8
        over the remaining 2048  -> candB (bf16).
      - cols [8192, 9216): VectorE max8 straight from PSUM -> candA (fp32).
    Host merges the two candidate lists."""
    from contextlib import ExitStack
    from concourse import bacc, tile, mybir

    nc = bacc.Bacc("TRN2", target_bir_lowering=False, debug=False,
                   enable_asserts=False)

    lhsT_d = nc.dram_tensor("lhsT", [KDIM, ROWS], mybir.dt.bfloat16,
                            kind="ExternalInput").ap()
    rhs_d = nc.dram_tensor("rhs", [KDIM, N], mybir.dt.bfloat16,
                           kind="ExternalInput").ap()
    candA_d = nc.dram_tensor("candA", [ROWS, 2 * TOPK], mybir.dt.float32,
                             kind="ExternalOutput").ap()
    candB_d = nc.dram_tensor("candB", [ROWS, TOPK], mybir.dt.float16,
                             kind="ExternalOutput").ap()

    with tile.TileContext(nc) as tc:
        with ExitStack() as ctx:
            const = ctx.enter_context(tc.tile_pool(name="const", bufs=1))
            ev_psum_pool = ctx.enter_context(
                tc.tile_pool(name="ev_psum", bufs=2, space="PSUM"))
            dir_psum_pool = ctx.enter_context(
                tc.tile_pool(name="dir_psum", bufs=2, space="PSUM"))
            dist_pool = ctx.enter_context(tc.tile_pool(name="dist", bufs=3))
            f1_pool = ctx.enter_context(tc.tile_pool(name="f1", bufs=3))
            f2_pool = ctx.enter_context(tc.tile_pool(name="f2", bufs=2))
            cand_pool = ctx.enter_context(tc.tile_pool(name="cand", bufs=4))

            qT = const.tile([KDIM, ROWS], mybir.dt.bfloat16)
            pT = const.tile([KDIM, N], mybir.dt.bfloat16)
            # ordered + split across two trigger queues so tile 0's
            # operands land first: sync carries qT-head + the direct-part
            # columns, gpsimd carries the bulk
            nc.sync.dma_start(qT[:, :TILE_P], lhsT_d[:, :TILE_P])
            nc.sync.dma_start(pT[:, N_EVICT:], rhs_d[:, N_EVICT:])
            nc.gpsimd.dma_start(qT[:, TILE_P:], lhsT_d[:, TILE_P:])
            for c in range(0, N_EVICT, 1792):
                nc.sync.dma_start(pT[:, c:c + 1792], rhs_d[:, c:c + 1792])

            DIR_CHUNK = N_DIRECT // 2

            def emit_direct(t, lhs_tile):
                # direct part: cols [N_EVICT, N) -> max8 straight from PSUM,
                # as two double-buffered 2-bank tiles so PE is never blocked
                # behind a pending max8.
                for j in range(2):
                    psum_dir = dir_psum_pool.tile([TILE_P, DIR_CHUNK],
                                                  mybir.dt.float32, tag="dir")
                    c0 = N_EVICT + j * DIR_CHUNK
                    for b in range(0, DIR_CHUNK, MM_N):
                        nc.tensor.matmul(
                            psum_dir[:, b:b + MM_N],
                            lhs_tile,
                            pT[:, c0 + b:c0 + b + MM_N],
                            start=True, stop=True)
                    candA = cand_pool.tile([TILE_P, TOPK], mybir.dt.float32,
                                           tag="candA")
                    nc.vector.max(out=candA[:], in_=psum_dir[:])
                    nc.sync.dma_start(
                        candA_d[t * TILE_P:(t + 1) * TILE_P,
                                j * TOPK:(j + 1) * TOPK],
                        candA[:])

            for t in range(N_TILES):
                lhs_tile = qT[:, t * TILE_P:(t + 1) * TILE_P]

                if t == 0:
                    # tile 0: direct part first so VectorE has work while
                    # the eviction pipeline fills
                    emit_direct(t, lhs_tile)

                # evicted part: cols [0, N_EVICT), ScalarE drains PSUM->fp16
                dist = dist_pool.tile([TILE_P, N_EVICT], mybir.dt.float16)
                for c in range(0, N_EVICT, EV_CHUNK):
                    psum = ev_psum_pool.tile([TILE_P, EV_CHUNK],
                                             mybir.dt.float32, tag="ps")
                    for b in range(0, EV_CHUNK, MM_N):
                        nc.tensor.matmul(
                            psum[:, b:b + MM_N],
                            lhs_tile,
                            pT[:, c + b:c + b + MM_N],
                            start=True, stop=True)
                    nc.scalar.activation(
                        dist[:, c:c + EV_CHUNK], psum[:],
                        mybir.ActivationFunctionType.Copy)

                if t > 0:
                    emit_direct(t, lhs_tile)

                candB = cand_pool.tile([TILE_P, TOPK], mybir.dt.float16,
                                       tag="candB")
                # chunk-pair fold tree: starts as soon as two chunks are
                # evicted; max 4 source columns per folded slot (same
                # collision budget as a half-fold chain), contiguous 2048
                # final max8, and only one small TT + the max8 after the
                # final eviction.
                ck = EV_CHUNK
                tr = f1_pool.tile([TILE_P, 4 * ck], mybir.dt.float16,
                                  tag="tree")
                nc.vector.tensor_max(tr[:, 2 * ck:3 * ck],
                                     dist[:, 0:ck], dist[:, ck:2 * ck])
                nc.vector.tensor_max(tr[:, 3 * ck:4 * ck],
                                     dist[:, 2 * ck:3 * ck],
                                     dist[:, 3 * ck:4 * ck])
                nc.vector.tensor_max(tr[:, 0:ck],
                                     tr[:, 2 * ck:3 * ck],
                                     tr[:, 3 * ck:4 * ck])
                nc.vector.tensor_max(tr[:, 2 * ck:3 * ck],
                                     dist[:, 4 * ck:5 * ck],
                                     dist[:, 5 * ck:6 * ck])
                nc.vector.tensor_max(tr[:, ck:2 * ck],
                                     tr[:, 2 * ck:3 * ck],
                                     dist[:, 6 * ck:7 * ck])
                nc.vector.max(out=candB[:], in_=tr[:, 0:2 * ck])
                nc.sync.dma_start(candB_d[t * TILE_P:(t + 1) * TILE_P, :],
                                  candB[:])

    nc.compile()
    return nc


def _split_hi_lo(x32):
    """fp32 array -> (hi, lo) bf16 pair with hi + lo ~= x to ~18 bits."""
    hi = x32.astype(BF16)
    lo = (x32 - hi.astype(np.float32)).astype(BF16)
    return hi, lo


def _prep_batch(p):
    """p: [N, 3] float32 pixels -> (lhsT_full [16, N], rhs [16, N]) bf16."""
    ph, pl = _split_hi_lo(p)                      # [N, 3] each
    p64 = ph.astype(np.float64) + pl.astype(np.float64)
    sqn = np.einsum("nd,nd->n", p64, p64)         # [N] float64
    snh = sqn.astype(BF16)
    snl = (sqn - snh.astype(np.float64)).astype(np.float32).astype(BF16)

    rhs = np.empty((KDIM, N), BF16)
    lhsT = np.empty((KDIM, N), BF16)
    for d in range(C):
        two_ph = (2.0 * ph[:, d].astype(np.float32)).astype(BF16)
        two_pl = (2.0 * pl[:, d].astype(np.float32)).astype(BF16)
        rhs[4 * d + 0] = two_ph
        rhs[4 * d + 1] = two_pl
        rhs[4 * d + 2] = two_ph
        rhs[4 * d + 3] = two_pl
        lhsT[4 * d + 0] = ph[:, d]
        lhsT[4 * d + 1] = ph[:, d]
        lhsT[4 * d + 2] = pl[:, d]
        lhsT[4 * d + 3] = pl[:, d]
    one = np.ones(N, BF16)
    rhs[12] = -snh
    rhs[13] = -snl
    rhs[14] = one
    rhs[15] = one
    lhsT[12] = one
    lhsT[13] = one
    lhsT[14] = -snh
    lhsT[15] = -snl
    return lhsT, rhs


def _enable_tracing():
    """Best-effort NTFF tracing under axon: install the missing
    antenv.axon_hooks shim and disable the artifact upload."""
    import sys
    import types
    try:
        import antenv.axon_hooks  # noqa: F401
    except ImportError:
        try:
            import antenv
            from trn_agent_boot.trn_boot import _ntff_profile_via_ctypes
            hook = _ntff_profile_via_ctypes("/opt/axon/libaxon_pjrt.so")
            mod = types.ModuleType("antenv.axon_hooks")
            state = {"hook": hook}
            mod.get_axon_ntff_profile_hook = lambda: state["hook"]
            mod.set_axon_ntff_profile_hook = (
                lambda h: state.__setitem__("hook", h))
            sys.modules["antenv.axon_hooks"] = mod
            antenv.axon_hooks = mod
        except Exception as e:  # tracing is optional
            print(f"tracing hook unavailable: {e}")
            return False
    from concourse import bass_utils
    bass_utils.upload_artifacts = lambda tmpdir: f"local://{tmpdir}"
    return True


def kernel(generated) -> np.ndarray:
    global LAST_RESULTS
    from concourse.bass_utils import run_bass_kernel_spmd

    if "nc" not in _CACHE:
        _CACHE["nc"] = _build_program()
    nc = _CACHE["nc"]

    g = np.asarray(generated).astype(np.float32)
    assert g.shape == (B, C, 96, 96), g.shape
    pixels = g.reshape(B, C, N).transpose(0, 2, 1)  # [B, N, 3]

    per_batch = [_prep_batch(np.ascontiguousarray(pixels[b]))
                 for b in range(B)]

    in_maps = []
    for core in range(N_CORES):
        b, ch = divmod(core, CHUNKS)
        lhsT_full, rhs = per_batch[b]
        in_maps.append({
            "lhsT": np.ascontiguousarray(
                lhsT_full[:, ch * ROWS:(ch + 1) * ROWS]),
            "rhs": rhs,
        })

    trace = bool(os.environ.get("KERNEL_TRACE"))
    if trace:
        trace = _enable_tracing()
    res = run_bass_kernel_spmd(
        nc, in_maps, list(range(N_CORES)),
        trace=trace,
        tmpdir=os.environ.get("KERNEL_TRACE_DIR") or None)
    LAST_RESULTS = res

    candA = np.stack([res.results[i]["candA"] for i in range(N_CORES)])
    candB = np.stack([res.results[i]["candB"].astype(np.float32)
                      for i in range(N_CORES)])
    # candA: [8, 2304, 16] (two direct chunks), candB: [8, 2304, 8]; all
    # -sq, descending per row.  Merge, take the global top 8 per row;
    # slot 0 is the diagonal (true value 0).
    cand = np.concatenate([candA, candB], axis=2)          # [8, 2304, 24]
    cand = -np.sort(-cand.astype(np.float64), axis=2)[:, :, :TOPK]
    sq = np.maximum(-cand, 0.0)
    d = np.sqrt(sq)
    total = d[:, :, 1:TOPK].sum()   # diagonal contributes exactly 0
    mean = total / (B * N * TOPK)
    return np.float32(-mean)
